# revision 1
# baseline (speedup 1.0000x reference)
"""2-layer GAT (GATNet) forward on 8 Trainium2 NeuronCores via Bass/Tile.

Sharding: 128 graphs -> 16 per core (graph-data parallel by destination so
pooling stays local). Each graph gets L padded slots (L = max graph size,
rounded) so all cores run one identical SPMD program.

Layer 1: every core computes h_ext = x @ [W1 | W1@att_src1 | W1@att_dst1]
for ALL nodes (fp32r matmul, replicated) and stores rows (h bf16 + a_src
f32) in a DRAM table hx plus a_dst in a small table at1. Each core
processes only edges whose dst is in its slots: dma_gather pulls h[src]
rows (one edge per SBUF partition) and a_dst[dst] rows;
ex = exp(leakyrelu(a_src+a_dst)) is computed per edge; messages are scaled
in place; a 0/1 selection matrix S[e, dst_local] (iota + is_equal) turns
the per-128-dst-window segmented softmax sum (numerator AND denominator)
into PE matmuls accumulated in PSUM. Normalize + bias + ELU per window.

Layer 2: h2_ext = elu1 @ [W2 | W2@att_src2 | W2@att_dst2] on local slots,
AllGather across cores, same edge pass with a single head.

Pooling: phantom slots masked to -1e30, one tensor_reduce(max) over the
[128, 16, L] view; FC + ReLU; each core outputs its 16 graphs [16, 128].
"""
import sys
import numpy as np

for _p in ("/opt/trn_rl_repo", "/root/.axon_site/_ro/trn_rl_repo"):
    if _p not in sys.path:
        sys.path.append(_p)

import json as _json
from contextlib import ExitStack

import concourse.bass as bass
import concourse.mybir as mybir
import concourse.tile as tile
import bass_rust as _bass_rust
import concourse.bass_utils as _bass_utils
import concourse.bass2jax as _bass2jax
from concourse.library_config import all_libraries as _all_libs, standard as _std_lib

F32 = mybir.dt.float32
F32R = mybir.dt.float32r
BF16 = mybir.dt.bfloat16
I16 = mybir.dt.int16
AF = mybir.ActivationFunctionType
OP = mybir.AluOpType

NC = 8
NEG_SLOPE = 0.2
EPS = 1e-6
NEG_BIG = -1.0e30
CH = 8           # gather chunk size in 128-edge blocks
DMA_SCRATCH = 16384   # SWDGE descriptor carveout: //16 = 1024 descriptors

# ------------------------------------------------------------- walrus fixups

_orig_compile_bir_kernel = _bass_utils.compile_bir_kernel


def _split_multiwaits(j):
    """This walrus build encodes at most ONE sync-wait per instruction;
    move extra waits onto NoOp carriers."""
    n = 0
    for f in j.get("functions", []):
        for bb in f.get("blocks", []):
            insts = bb.get("instructions", [])
            if not any(
                len(((i.get("sync_info") or {}).get("on_wait") or [])) > 1
                for i in insts
            ):
                continue
            new = []
            for i in insts:
                si = i.get("sync_info")
                w = (si or {}).get("on_wait") or []
                if len(w) > 1:
                    for extra in w[:-1]:
                        n += 1
                        new.append({
                            "debug": i.get("debug", 0),
                            "engine": i["engine"],
                            "ins": [], "outs": [],
                            "name": f"I-mws-{n}",
                            "opcode": "NoOp",
                            "sync_info": {"on_update": [], "on_wait": [extra]},
                        })
                    si["on_wait"] = [w[-1]]
                new.append(i)
            bb["instructions"] = new
    return j


def _patched_compile_bir_kernel(bir_json, tmpdir, neff_name="file.neff"):
    j = _json.loads(bir_json)
    j = _split_multiwaits(j)
    return _orig_compile_bir_kernel(
        _json.dumps(j).encode(), tmpdir, neff_name=neff_name)


def apply_patches():
    _bass_utils.compile_bir_kernel = _patched_compile_bir_kernel
    _bass2jax.compile_bir_kernel = _patched_compile_bir_kernel


def finalize_program(nc):
    """Bacc-style post passes that raw Bass/Tile skips: insert gpsimd
    library loads and encode extended-ISA instruction words."""
    mask = {}
    for lib in _all_libs:
        for it in lib.instructions:
            mask[it] = mask.get(it, 0) | (1 << lib.index)
    _bass_rust.insert_library_loads(nc, mask, len(_all_libs), _std_lib.index)
    mybir.codegen_inst_isa_subclasses(nc)


# ------------------------------------------------------------- host prep

def _wrap_idx(idx):
    """dma_gather idx layout: idx i -> partition i%16, slot i//16,
    replicated across the 8 groups of 16 partitions. [n] -> [128, n//16]."""
    n = len(idx)
    assert n % 16 == 0
    w = idx.reshape(n // 16, 16).T.astype(np.int16)
    return np.tile(w, (8, 1)).copy()


def host_prep(x, edge_index, batch):
    N, F = x.shape
    G = int(np.asarray(batch).max()) + 1
    assert G % NC == 0, f"graphs {G} not divisible by {NC}"
    GPC = G // NC

    src = np.concatenate([np.asarray(edge_index[0], np.int64),
                          np.arange(N, dtype=np.int64)])
    dst = np.concatenate([np.asarray(edge_index[1], np.int64),
                          np.arange(N, dtype=np.int64)])

    bat = np.asarray(batch, dtype=np.int64)
    counts = np.bincount(bat, minlength=G)
    start = np.zeros(G + 1, dtype=np.int64)
    np.cumsum(counts, out=start[1:])

    stepmod = 128 // int(np.gcd(GPC, 128))
    L = int(np.ceil(max(1, counts.max()) / stepmod) * stepmod)
    SL = GPC * L
    W = SL // 128
    assert SL % 128 == 0
    assert G * L + 1 <= 32766, f"slot rows {G * L} overflow int16"
    assert N + 1 <= 32766

    # permute graphs: serpentine-deal by edge count so the k-th graph of
    # every core has a similar profile -> less per-window max padding
    ecnt = np.bincount(bat[dst], minlength=G)
    order = np.argsort(-ecnt, kind="stable")
    perm = np.zeros(G, dtype=np.int64)     # perm[c*GPC+k] = graph id
    gslot = np.zeros(G, dtype=np.int64)    # graph id -> c*GPC+k
    for i, g in enumerate(order):
        r, pos = divmod(i, NC)
        c = pos if (r % 2 == 0) else NC - 1 - pos
        perm[c * GPC + r] = g
        gslot[g] = c * GPC + r

    rank = np.arange(N, dtype=np.int64) - start[bat]
    slot_row = gslot[bat] * L + rank       # global slot row = core*SL + local

    e_core = gslot[bat[dst]] // GPC
    e_slot = slot_row[dst] - e_core * SL   # local dst slot on owning core
    e_w = e_slot // 128

    order = np.lexsort((e_w, e_core))
    src_s, dst_s = src[order], dst[order]
    core_s, w_s, eslot_s = e_core[order], e_w[order], e_slot[order]

    cnt = np.zeros((NC, W), dtype=np.int64)
    np.add.at(cnt, (core_s, w_s), 1)
    B = np.maximum(1, (cnt.max(axis=0) + 127) // 128)
    TB = int(B.sum())
    NEP = TB * 128

    l1src = np.full((NC, NEP), N, dtype=np.int64)
    l1dst = np.full((NC, NEP), N, dtype=np.int64)
    l2src = np.zeros((NC, NEP), dtype=np.int64)
    l2dstloc = np.full((NC, NEP), SL, dtype=np.int64)
    dloc = np.zeros((NC, NEP), dtype=np.float32)

    w_off = np.zeros(W + 1, dtype=np.int64)
    np.cumsum(B * 128, out=w_off[1:])

    flat = core_s * W + w_s
    rs = np.searchsorted(flat, np.arange(NC * W))
    re = np.searchsorted(flat, np.arange(NC * W) + 1)
    for c in range(NC):
        for w in range(W):
            a, b = rs[c * W + w], re[c * W + w]
            n = b - a
            o = w_off[w]
            l1src[c, o:o + n] = src_s[a:b]
            l1dst[c, o:o + n] = dst_s[a:b]
            l2src[c, o:o + n] = slot_row[src_s[a:b]]
            l2dstloc[c, o:o + n] = eslot_s[a:b]
            dloc[c, o:o + n] = (eslot_s[a:b] % 128).astype(np.float32)

    chunks = []
    b0 = 0
    while b0 < TB:
        nb = min(CH, TB - b0)
        chunks.append((b0, nb))
        b0 += nb

    def build_wrapped(arr):
        parts = []
        for (cb0, nb) in chunks:
            parts.append(_wrap_idx(arr[cb0 * 128:(cb0 + nb) * 128]))
        return np.concatenate(parts, axis=1)

    ph = np.full((NC, SL), NEG_BIG, dtype=np.float32)
    for c in range(NC):
        for k in range(GPC):
            g = perm[c * GPC + k]
            ph[c, k * L:k * L + counts[g]] = 0.0

    return dict(
        N=N, F=F, G=G, GPC=GPC, L=L, SL=SL, W=W, TB=TB, perm=perm,
        B=[int(b) for b in B], chunks=chunks,
        l1src_w=np.stack([build_wrapped(l1src[c]) for c in range(NC)]),
        l1dst_w=np.stack([build_wrapped(l1dst[c]) for c in range(NC)]),
        l2src_w=np.stack([build_wrapped(l2src[c]) for c in range(NC)]),
        l2dst_w=np.stack([build_wrapped(l2dstloc[c]) for c in range(NC)]),
        dloc_t=np.stack([dloc[c].reshape(TB, 128).T.copy()
                         for c in range(NC)]),
        ph_t=np.stack([ph[c].reshape(W, 128).T.copy() for c in range(NC)]),
    )


# ------------------------------------------------------------- program

def build_program(meta, H, D, D2, use_f32r=True):
    N, F, G = meta["N"], meta["F"], meta["G"]
    GPC, L, SL, W, TB = meta["GPC"], meta["L"], meta["SL"], meta["W"], meta["TB"]
    B, chunks = meta["B"], meta["chunks"]
    assert F <= 128 and D == 128

    HD = H * D
    N1 = HD + 2 * H                              # phase-B output cols
    RS1 = ((HD + 2 * H + 127) // 128) * 128      # hx row stride/elem (units)
    ND1 = HD + H                                 # scatter cols (msg | ex)
    NB1 = [(k * 512, min((k + 1) * 512, ND1)) for k in range((ND1 + 511) // 512)]
    NBB = [(k * 512, min((k + 1) * 512, N1)) for k in range((N1 + 511) // 512)]
    KD = HD // 128
    assert HD % 128 == 0
    N2 = D2 + 2
    RS2 = ((D2 + 2 + 127) // 128) * 128          # h2x row stride/elem (units)
    ND2 = D2 + 1
    NROW2 = G * L
    nblk = (N + 127) // 128

    nc = bass.Bass(dynamic_dma_scratch_size=DMA_SCRATCH)

    xT_d = nc.declare_dram_parameter("xT", [F, N], BF16, isOutput=False)
    W1_d = nc.declare_dram_parameter("W1", [F, HD], F32, isOutput=False)
    W1T_d = nc.declare_dram_parameter("W1T", [HD, F], F32, isOutput=False)
    att1T_d = nc.declare_dram_parameter("att1T", [D, 2 * H], F32, isOutput=False)
    b1_d = nc.declare_dram_parameter("b1", [1, HD], F32, isOutput=False)
    W2_d = nc.declare_dram_parameter("W2", [HD, D2], F32, isOutput=False)
    W2T_d = nc.declare_dram_parameter("W2T", [D2, HD], F32, isOutput=False)
    att2T_d = nc.declare_dram_parameter("att2T", [D2, 2], F32, isOutput=False)
    b2_d = nc.declare_dram_parameter("b2", [1, D2], F32, isOutput=False)
    fcW_d = nc.declare_dram_parameter("fcW", [D2, D2], F32, isOutput=False)
    fcb_d = nc.declare_dram_parameter("fcb", [1, D2], F32, isOutput=False)
    iota_d = nc.declare_dram_parameter("iota128", [128, 128], BF16, isOutput=False)
    idbf_d = nc.declare_dram_parameter("idbf", [128, 128], BF16, isOutput=False)
    idf32_d = nc.declare_dram_parameter("idf32", [128, 128], F32, isOutput=False)
    l1src_d = nc.declare_dram_parameter("l1src", [128, TB * 8], I16, isOutput=False)
    l1dst_d = nc.declare_dram_parameter("l1dst", [128, TB * 8], I16, isOutput=False)
    l2src_d = nc.declare_dram_parameter("l2src", [128, TB * 8], I16, isOutput=False)
    l2dst_d = nc.declare_dram_parameter("l2dst", [128, TB * 8], I16, isOutput=False)
    dloc_d = nc.declare_dram_parameter("dloc", [128, TB], F32, isOutput=False)
    ph_d = nc.declare_dram_parameter("phmask", [128, W], F32, isOutput=False)
    out_d = nc.declare_dram_parameter("out", [GPC, D2], F32, isOutput=True)

    with tile.TileContext(nc) as tc, ExitStack() as ctx:
        dram = ctx.enter_context(tc.tile_pool(name="dram", bufs=1, space="DRAM"))
        hx = dram.tile([N + 1, RS1], BF16)
        at1 = dram.tile([N + 1, 64], F32)
        adst2 = dram.tile([SL + 1, 64], F32)
        elu1d = dram.tile([SL, HD], BF16)
        h2x_shard = dram.tile([SL, RS2], BF16)
        h2x = dram.tile([NROW2, RS2], BF16, addr_space="Shared")

        const = ctx.enter_context(tc.tile_pool(name="const", bufs=1))
        res = ctx.enter_context(tc.tile_pool(name="res", bufs=1))

        iota_f = const.tile([128, 128], BF16)
        nc.sync.dma_start(out=iota_f[:], in_=iota_d[:])
        idbf = const.tile([128, 128], BF16)
        nc.sync.dma_start(out=idbf[:], in_=idbf_d[:])
        idf32 = const.tile([128, 128], F32)
        nc.sync.dma_start(out=idf32[:], in_=idf32_d[:])
        dloc_t = const.tile([128, TB], F32)
        nc.sync.dma_start(out=dloc_t[:], in_=dloc_d[:])
        ph_t = const.tile([128, W], F32)
        nc.sync.dma_start(out=ph_t[:], in_=ph_d[:])

        b1bc = const.tile([128, HD], BF16)
        b2row = const.tile([1, D2], F32)
        nc.sync.dma_start(out=b2row[:], in_=b2_d[:])
        b2bc = const.tile([128, D2], F32)
        nc.gpsimd.partition_broadcast(b2bc[:], b2row[:])
        fcbrow = const.tile([1, D2], F32)
        nc.sync.dma_start(out=fcbrow[:], in_=fcb_d[:])
        fcbbc = const.tile([128, D2], F32)
        nc.gpsimd.partition_broadcast(fcbbc[:], fcbrow[:])
        fcw_t = const.tile([D2, D2], F32)
        nc.sync.dma_start(out=fcw_t[:], in_=fcW_d[:])

        w2ext = res.tile([128, KD, D2 + 2], BF16)
        out2T = res.tile([128, SL], F32)

        # ---------------- phase A: Wext = [W1 | W1@att_src1 | W1@att_dst1]
        pA = ctx.enter_context(tc.tile_pool(name="phA", bufs=1))
        with tc.tile_pool(name="psA", bufs=1, space="PSUM") as psA:
            b1row = pA.tile([1, HD], F32)
            nc.sync.dma_start(out=b1row[:], in_=b1_d[:])
            b1bcf = pA.tile([128, HD], F32)
            nc.gpsimd.partition_broadcast(b1bcf[:], b1row[:])
            nc.vector.tensor_copy(b1bc[:], b1bcf[:])
            wext = pA.tile([F, N1], BF16)
            w1f = pA.tile([F, HD], F32)
            nc.sync.dma_start(out=w1f[:], in_=W1_d[:])
            nc.vector.tensor_copy(wext[:, 0:HD], w1f[:])
            w1t_t = pA.tile([128, H, F], F32)
            for h in range(H):
                nc.sync.dma_start(out=w1t_t[:, h, :],
                                  in_=W1T_d[h * 128:(h + 1) * 128, :])
            att1t_t = pA.tile([D, 2 * H], F32)
            nc.sync.dma_start(out=att1t_t[:], in_=att1T_d[:])
            watt_ps = psA.tile([F, 2 * H], F32)
            for h in range(H):
                nc.tensor.matmul(out=watt_ps[:, h:h + 1],
                                 lhsT=w1t_t[:, h, :],
                                 rhs=att1t_t[:, h:h + 1],
                                 start=True, stop=True)
                nc.tensor.matmul(out=watt_ps[:, H + h:H + h + 1],
                                 lhsT=w1t_t[:, h, :],
                                 rhs=att1t_t[:, H + h:H + h + 1],
                                 start=True, stop=True)
            nc.vector.tensor_copy(wext[:, HD:HD + 2 * H], watt_ps[:])

            w2t_t = pA.tile([D2, HD], F32)
            nc.sync.dma_start(out=w2t_t[:], in_=W2T_d[:])
            att2t_t = pA.tile([D2, 2], F32)
            nc.sync.dma_start(out=att2t_t[:], in_=att2T_d[:])
            for j in range(KD):
                w2c = pA.tile([128, D2], F32, tag="w2c")
                nc.sync.dma_start(out=w2c[:],
                                  in_=W2_d[j * 128:(j + 1) * 128, :])
                nc.vector.tensor_copy(w2ext[:, j, 0:D2], w2c[:])
                w2a_ps = psA.tile([128, 2], F32, tag="w2a")
                nc.tensor.matmul(out=w2a_ps[:],
                                 lhsT=w2t_t[:, j * 128:(j + 1) * 128],
                                 rhs=att2t_t[:], start=True, stop=True)
                nc.vector.tensor_copy(w2ext[:, j, D2:D2 + 2], w2a_ps[:])

            padrow = pA.tile([1, RS1], BF16)
            nc.vector.memset(padrow[:], 0.0)
            nc.vector.memset(padrow[:, HD:HD + 2 * H].bitcast(F32), NEG_BIG)
            nc.sync.dma_start(out=hx[N:N + 1, :], in_=padrow[:])
            padat = pA.tile([1, 64], F32)
            nc.vector.memset(padat[:], 0.0)
            nc.sync.dma_start(out=at1[N:N + 1, :], in_=padat[:])
            padad2 = pA.tile([1, 64], F32)
            nc.vector.memset(padad2[:], NEG_BIG)
            nc.sync.dma_start(out=adst2[SL:SL + 1, :], in_=padad2[:])

        # ---------------- phase B: h_ext for all nodes (replicated)
        with tc.tile_pool(name="xT", bufs=1) as pxT, \
             tc.tile_pool(name="phB", bufs=6) as pB, \
             tc.tile_pool(name="psB", bufs=2, space="PSUM") as psB:
            xT_t = pxT.tile([F, N], BF16)
            nc.sync.dma_start(out=xT_t[:], in_=xT_d[:])
            for nb in range(nblk):
                r0 = nb * 128
                rn = min(128, N - r0)
                hps = psB.tile([128, N1], F32, tag="hps")
                for (c0, c1) in NBB:
                    nc.tensor.matmul(
                        out=hps[0:rn, c0:c1],
                        lhsT=xT_t[:, r0:r0 + rn],
                        rhs=wext[:, c0:c1],
                        start=True, stop=True)
                hrow = pB.tile([128, RS1], BF16, tag="hrow")
                if HD + 2 * H < RS1:
                    nc.vector.memset(hrow[0:rn, HD + 2 * H:RS1], 0.0)
                nc.scalar.copy(hrow[0:rn, 0:HD], hps[0:rn, 0:HD])
                nc.vector.tensor_copy(
                    hrow[0:rn, HD:HD + 2 * H].bitcast(F32),
                    hps[0:rn, HD:HD + H])
                nc.sync.dma_start(out=hx[r0:r0 + rn, :], in_=hrow[0:rn, :])
                jb = nb % 4
                if jb == 0:
                    atrow = pB.tile([128, 4, 64], F32, tag="atrow", name="atrow")
                nc.vector.memset(atrow[0:rn, jb, H:64], 0.0)
                nc.scalar.copy(atrow[0:rn, jb, 0:H],
                               hps[0:rn, HD + H:HD + 2 * H])
                if jb == 3 or nb == nblk - 1:
                    nj = jb + 1
                    a0 = (nb - jb) * 128
                    arows = min(4 * 128, N - a0)
                    dst_ap = bass.AP(at1.tensor, at1[:].offset + a0 * 64,
                                     [[64, min(128, arows)], [128 * 64, nj],
                                      [1, 64]])
                    nc.sync.dma_start(out=dst_ap, in_=atrow[0:min(128, arows),
                                                            0:nj, :])

        # ---------------- edge pass (shared between the two layers)
        _nreg_cache = {}

        def nreg(v):
            if v not in _nreg_cache:
                _nreg_cache[v] = nc.gpsimd.to_reg(v)
            return _nreg_cache[v]

        blk_win = []
        for w in range(W):
            for i in range(B[w]):
                blk_win.append((w, i))

        def edge_pass(layer):
            if layer == 1:
                table, idxd, adt, adidxd = hx, l1src_d, at1, l1dst_d
                ELEM, nd, heads, hd, nbch = RS1, ND1, H, HD, NB1
            else:
                table, idxd, adt, adidxd = h2x, l2src_d, adst2, l2dst_d
                ELEM, nd, heads, hd, nbch = RS2, ND2, 1, D2, [(0, ND2)]

            with tc.tile_pool(name=f"gth{layer}", bufs=4) as pG, \
                 tc.tile_pool(name=f"chn{layer}", bufs=2) as pC2, \
                 tc.tile_pool(name=f"spool{layer}", bufs=4) as pS, \
                 tc.tile_pool(name=f"psw{layer}", bufs=2, space="PSUM") as psW, \
                 tc.tile_pool(name=f"pst{layer}", bufs=2, space="PSUM") as psT, \
                 tc.tile_pool(name=f"nrm{layer}", bufs=2) as pN, \
                 tc.tile_pool(name=f"idx{layer}", bufs=1) as pI:

                idxt = pI.tile([128, TB * 8], I16, name="idxt")
                nc.sync.dma_start(out=idxt[:], in_=idxd[:])
                adidxt = pI.tile([128, TB * 8], I16, name="adidxt")
                nc.sync.dma_start(out=adidxt[:], in_=adidxd[:])

                state = {"w": -1, "ps": None}

                def normalize():
                    w, win_ps = state["w"], state["ps"]
                    rec = pN.tile([128, heads], F32, tag="rec")
                    nc.vector.tensor_scalar_add(rec[:], win_ps[:, hd:hd + heads],
                                                EPS)
                    nc.vector.reciprocal(rec[:], rec[:])
                    odt = BF16 if layer == 1 else F32
                    o1 = pN.tile([128, hd], odt, tag="o1")
                    for h in range(heads):
                        nc.scalar.activation(
                            o1[:, h * D:(h + 1) * D],
                            win_ps[:, h * D:(h + 1) * D],
                            AF.Copy, scale=rec[:, h:h + 1])
                    bt = b1bc if layer == 1 else b2bc
                    t1 = pN.tile([128, hd], odt, tag="t1")
                    nc.vector.tensor_tensor(t1[:], o1[:], bt[:], OP.add)
                    t2 = pN.tile([128, hd], odt, tag="t2")
                    nc.vector.tensor_scalar_min(t2[:], t1[:], 0.0)
                    e1 = pN.tile([128, hd], odt, tag="e1")
                    nc.scalar.activation(e1[:], t2[:], AF.Exp)
                    r1 = pN.tile([128, hd], odt, tag="r1")
                    nc.scalar.activation(r1[:], t1[:], AF.Relu)
                    el = pN.tile([128, hd], odt, tag="el")
                    nc.vector.scalar_tensor_tensor(
                        out=el[:], in0=e1[:], scalar=-1.0, in1=r1[:],
                        op0=OP.add, op1=OP.add)
                    if layer == 1:
                        nc.sync.dma_start(
                            out=elu1d[w * 128:(w + 1) * 128, :], in_=el[:])
                    else:
                        elm = pN.tile([128, hd], F32, tag="elm")
                        nc.vector.tensor_scalar_add(elm[:], el[:],
                                                    ph_t[:, w:w + 1])
                        tp = psT.tile([128, 128], F32, tag="tp")
                        nc.tensor.transpose(tp[:], elm[:], idf32[:])
                        nc.vector.tensor_copy(out2T[:, w * 128:(w + 1) * 128],
                                              tp[:])

                idx_off = 0
                for (cb0, nbk) in chunks:
                    ne = nbk * 128
                    gt = pG.tile([128, CH, ELEM], BF16, tag="gt")
                    nc.gpsimd.dma_gather(
                        out_ap=gt[:, 0:nbk, :],
                        in_ap=table[:, 0:ELEM],
                        idxs_ap=idxt[:, idx_off:idx_off + nbk * 8],
                        num_idxs=ne, num_idxs_reg=nreg(ne), elem_size=ELEM)
                    ad = pG.tile([128, CH, 64], F32, tag="ad")
                    nc.gpsimd.dma_gather(
                        out_ap=ad[:, 0:nbk, :],
                        in_ap=adt[:],
                        idxs_ap=adidxt[:, idx_off:idx_off + nbk * 8],
                        num_idxs=ne, num_idxs_reg=nreg(ne), elem_size=64)
                    idx_off += nbk * 8

                    # e = a_src + a_dst ; leakyrelu ; exp  (batched per chunk)
                    if layer == 1:
                        asrc = gt[:, 0:nbk, HD:HD + 2 * H].bitcast(F32)
                    else:
                        asrc = gt[:, 0:nbk, D2:D2 + 2].bitcast(F32)
                    et = pC2.tile([128, CH * heads], F32, tag="et")
                    nc.vector.tensor_tensor(
                        et[:, 0:nbk * heads], asrc, ad[:, 0:nbk, 0:heads],
                        OP.add)
                    lk = pC2.tile([128, CH * heads], F32, tag="lk")
                    nc.vector.scalar_tensor_tensor(
                        out=lk[:, 0:nbk * heads], in0=et[:, 0:nbk * heads],
                        scalar=NEG_SLOPE, in1=et[:, 0:nbk * heads],
                        op0=OP.mult, op1=OP.max)
                    exf = pC2.tile([128, CH * heads], F32, tag="exf")
                    nc.scalar.activation(exf[:, 0:nbk * heads],
                                         lk[:, 0:nbk * heads], AF.Exp)
                    exb = pC2.tile([128, CH, heads], BF16, tag="exb")
                    nc.vector.tensor_copy(exb[:, 0:nbk, :],
                                          exf[:, 0:nbk * heads])

                    # scale messages in place, append ex columns
                    msg4 = bass.AP(gt.tensor, gt[:].offset,
                                   [gt[:].ap[0], [ELEM, nbk], [D, heads],
                                    [1, D]])
                    exb4 = bass.AP(exb.tensor, exb[:].offset,
                                   [exb[:].ap[0], [heads, nbk], [1, heads],
                                    [0, D]])
                    nc.vector.tensor_tensor(msg4, msg4, exb4, OP.mult)
                    nc.vector.tensor_copy(
                        bass.AP(gt.tensor, gt[:].offset + hd,
                                [gt[:].ap[0], [ELEM, nbk], [1, heads]]),
                        exb[:, 0:nbk, :])

                    # scatter matmuls per block
                    for i in range(nbk):
                        b = cb0 + i
                        w, pos = blk_win[b]
                        if w != state["w"]:
                            if state["w"] >= 0:
                                normalize()
                            state["w"] = w
                            state["ps"] = psW.tile([128, nd], F32, tag="winps", name="winps")
                        s_t = pS.tile([128, 128], BF16, tag="s")
                        nc.vector.tensor_scalar(
                            out=s_t[:], in0=iota_f[:],
                            scalar1=dloc_t[:, b:b + 1], scalar2=None,
                            op0=OP.is_equal)
                        first, last = pos == 0, pos == B[w] - 1
                        for (c0, c1) in nbch:
                            nc.tensor.matmul(
                                out=state["ps"][:, c0:c1],
                                lhsT=s_t[:],
                                rhs=gt[:, i, c0:c1],
                                start=first, stop=last)
                normalize()

        edge_pass(1)

        # ---------------- phase D: h2_ext = elu1 @ W2ext on my slots
        with tc.tile_pool(name="phD", bufs=4) as pD, \
             tc.tile_pool(name="psD", bufs=2, space="PSUM") as psD:
            for m in range(W):
                eld = pD.tile([128, HD], BF16, tag="eld")
                nc.sync.dma_start(out=eld[:],
                                  in_=elu1d[m * 128:(m + 1) * 128, :])
                elT = pD.tile([128, KD, 128], BF16, tag="elT")
                for j in range(KD):
                    tpj = psD.tile([128, 128], BF16, tag="tpj")
                    nc.tensor.transpose(tpj[:], eld[:, j * 128:(j + 1) * 128],
                                        idbf[:])
                    nc.vector.tensor_copy(elT[:, j, :], tpj[:])
                h2ps = psD.tile([128, N2], F32, tag="h2ps")
                for j in range(KD):
                    nc.tensor.matmul(
                        out=h2ps[:],
                        lhsT=elT[:, j, :],
                        rhs=w2ext[:, j, :],
                        start=(j == 0), stop=(j == KD - 1))
                row2 = pD.tile([128, RS2], BF16, tag="row2")
                nc.vector.memset(row2[:], 0.0)
                nc.vector.tensor_copy(row2[:, 0:D2], h2ps[:, 0:D2])
                nc.vector.tensor_copy(
                    row2[:, D2:D2 + 2].bitcast(F32), h2ps[:, D2:D2 + 1])
                nc.sync.dma_start(out=h2x_shard[m * 128:(m + 1) * 128, :],
                                  in_=row2[:])
                adrow = pD.tile([128, 64], F32, tag="adrow")
                nc.vector.memset(adrow[:], 0.0)
                nc.vector.tensor_copy(adrow[:, 0:1], h2ps[:, D2 + 1:D2 + 2])
                nc.sync.dma_start(out=adst2[m * 128:(m + 1) * 128, :],
                                  in_=adrow[:])

            nc.gpsimd.collective_compute(
                "AllGather", OP.bypass,
                replica_groups=[list(range(NC))],
                ins=[h2x_shard[:]],
                outs=[h2x[0:NROW2, :]])


        # ---------------- phase E: layer-2 edge pass
        edge_pass(2)

        # ---------------- phase F: pooling + FC
        with tc.tile_pool(name="phF", bufs=1) as pF, \
             tc.tile_pool(name="psF", bufs=1, space="PSUM") as psF:
            pooled = pF.tile([128, GPC], F32)
            o2v = bass.AP(out2T.tensor, out2T[:].offset,
                          [out2T[:].ap[0], [L, GPC], [1, L]])
            nc.vector.tensor_reduce(pooled[:], o2v,
                                    axis=mybir.AxisListType.X, op=OP.max)
            fcps = psF.tile([GPC, D2], F32)
            nc.tensor.matmul(out=fcps[:], lhsT=pooled[:], rhs=fcw_t[:],
                             start=True, stop=True)
            fco = pF.tile([GPC, D2], F32)
            nc.vector.tensor_tensor(fco[:], fcps[:], fcbbc[0:GPC, :], OP.add)
            fcr = pF.tile([GPC, D2], F32)
            nc.scalar.activation(fcr[:], fco[:], AF.Relu)
            nc.sync.dma_start(out=out_d[:], in_=fcr[:])

    return nc


# ------------------------------------------------------------- entry point

def make_in_maps(meta, x, W1, att_src1, att_dst1, b1, W2, att_src2, att_dst2,
                 b2, fc_W, fc_b):
    import ml_dtypes
    shared = {
        "xT": np.ascontiguousarray(
            np.asarray(x, np.float32).T.astype(ml_dtypes.bfloat16)),
        "W1": np.asarray(W1, np.float32),
        "W1T": np.ascontiguousarray(np.asarray(W1, np.float32).T),
        "att1T": np.ascontiguousarray(np.concatenate(
            [np.asarray(att_src1, np.float32).T,
             np.asarray(att_dst1, np.float32).T], axis=1)),
        "b1": np.asarray(b1, np.float32).reshape(1, -1),
        "W2": np.asarray(W2, np.float32),
        "W2T": np.ascontiguousarray(np.asarray(W2, np.float32).T),
        "att2T": np.ascontiguousarray(np.concatenate(
            [np.asarray(att_src2, np.float32).T,
             np.asarray(att_dst2, np.float32).T], axis=1)),
        "b2": np.asarray(b2, np.float32).reshape(1, -1),
        "fcW": np.asarray(fc_W, np.float32),
        "fcb": np.asarray(fc_b, np.float32).reshape(1, -1),
        "iota128": np.tile(np.arange(128, dtype=ml_dtypes.bfloat16), (128, 1)),
        "idbf": np.eye(128, dtype=ml_dtypes.bfloat16),
        "idf32": np.eye(128, dtype=np.float32),
    }
    in_maps = []
    for c in range(NC):
        m = dict(shared)
        m["l1src"] = meta["l1src_w"][c]
        m["l1dst"] = meta["l1dst_w"][c]
        m["l2src"] = meta["l2src_w"][c]
        m["l2dst"] = meta["l2dst_w"][c]
        m["dloc"] = meta["dloc_t"][c]
        m["phmask"] = meta["ph_t"][c]
        in_maps.append(m)
    return in_maps


def kernel(**inputs):
    apply_patches()
    from concourse.bass_utils import run_bass_kernel_spmd

    x = np.asarray(inputs["x"], np.float32)
    att_src1 = np.asarray(inputs["att_src1"], np.float32)
    H, D = att_src1.shape
    D2 = np.asarray(inputs["W2"]).shape[1]

    meta = host_prep(x, inputs["edge_index"], inputs["batch"])
    nc = build_program(meta, H, D, D2)
    finalize_program(nc)
    in_maps = make_in_maps(
        meta, x, inputs["W1"], att_src1, inputs["att_dst1"], inputs["b1"],
        inputs["W2"], inputs["att_src2"], inputs["att_dst2"], inputs["b2"],
        inputs["fc_W"], inputs["fc_b"])
    res = run_bass_kernel_spmd(nc, in_maps, list(range(NC)))
    D2o = np.asarray(inputs["W2"]).shape[1]
    G = meta["G"]
    out = np.zeros((G, D2o), np.float32)
    for c in range(NC):
        rows = np.asarray(res.results[c]["out"])
        for k in range(meta["GPC"]):
            out[meta["perm"][c * meta["GPC"] + k]] = rows[k]
    return out

